# revision 49
# baseline (speedup 1.0000x reference)
"""Trainium2 Bass kernel for PixelUnshuffle->MHA->PixelShuffle (nn_Attention).

Reference computation (per batch element, 8 batch elements data-parallel
across 8 NeuronCores):
  x [64, 256, 256] --PixelUnshuffle(8)--> tokens [N=1024, C=4096]
  qkv = tokens @ W_qkv            [1024, 768]
  4-head attention (d=64), softmax over tokens
  y = attn_out @ W_out + b_out    [1024, 4096]
  --PixelShuffle(8)--> [64, 256, 256]

v4 structure: the exp-bound attention stage is dissolved into the
projection stages so the scalar engine's exp stream (the old stage-2
floor, ~37us) hides under PE matmul work:

  Phase A: qk projections for both token windows (PE-dense).  The
    window-0 attention logits (dotsT = kT x qT) + exps for n-half 0,
    m-chunks 0-3 are interleaved into the window-1 qk pass.
  Phase B: v projections for both windows with ALL remaining dots+exp
    pairs interleaved (paced ~1 pair per 1.2us of PE work so the
    in-order PE queue never stalls on the dt PSUM double-buffer).  The
    exp outputs (ed tiles, bf16) are buffered in SBUF (64KB/partition).
  Phase C: attn@v runs as a dense PE burst per (n-half, head-pair)
    block (no exp dependency left), each block's 1/Z normalize chain
    (SBUF redistribute DMA + reciprocal + DRAM-round-trip broadcast)
    overlaps later blocks, and the output projection starts as soon as
    outT[0] is normalized.

Attention math is unchanged from v3: dotsT[m, n] summed-token-on-
partitions, exp on scalar (the only exp engine), a ones column in v
accumulating the softmax denominator Z for free (row 64 of oaug).

Startup: the first x tile is split into 8 per-r2 SLICE TILES (tile
dependency tracking is tile-granular) so the first real matmul needs
only 128KB of x + one 256KB weight chunk (~11us) instead of the whole
1MB tile.  All queues are loaded in consumption order; aggregate early
HBM bandwidth is ~300 GB/s (8-core contention), which bounds the qk-w0
pass; the r2 consumption order [0,4,1,5,2,6,3,7] alternates the
scalar/gpsimd weight queues to match arrival.

Layout/host packing identical to v3: x pre-packed+pre-cast bf16 into
[w*4+cg, p, r2*512 + hh*32 + ww] tiles; W_qkv split qk/v columns in
exact SBUF chunk layout; output written as raw [nq, ct, p, hq, ww, r2]
bf16 tiles and pixel-shuffled + upcast on the host.
"""

import sys

if "/opt/trn_rl_repo" not in sys.path:
    sys.path.insert(0, "/opt/trn_rl_repo")

import os

import ml_dtypes
import numpy as np

import concourse.bass as bass
from concourse import bacc, mybir, tile
from concourse.bass_utils import run_bass_kernel_spmd

F32 = mybir.dt.float32
BF16 = mybir.dt.bfloat16

SCALE = 0.125  # DIM_HEAD ** -0.5

_CACHE = {}

R2_ORDER = [0, 4, 1, 5, 2, 6, 3, 7]


def _build(zero_bias=False):
    nc = bacc.Bacc("TRN2", target_bir_lowering=False, debug=False, num_devices=8)

    # x pre-packed on host: [w*4+cg, p, r2*512 + hh*32 + ww]
    x_d = nc.dram_tensor("x", [8, 128, 4096], BF16, kind="ExternalInput").ap()
    # W_qkv split on host into qk columns and v columns, each pre-packed in
    # exact SBUF chunk layout so every load is one fully contiguous DMA
    wq_d = nc.dram_tensor("W_qkv", [8, 2, 128, 1024], BF16, kind="ExternalInput").ap()
    wqv_d = nc.dram_tensor("W_qkv_v", [8, 2, 128, 512], BF16, kind="ExternalInput").ap()
    wo_d = nc.dram_tensor("W_out", [256, 4096], BF16, kind="ExternalInput").ap()
    b_d = nc.dram_tensor("b_out", [4096], F32, kind="ExternalInput").ap()
    # raw output tiles: [(nq*4+ct)*2+half, p, r2l*256 + hq*32 + ww] with
    # r2 = half*4 + r2l; host pixel-shuffles
    out_d = nc.dram_tensor("out", [32, 128, 1024], BF16, kind="ExternalOutput").ap()

    zrc_d = nc.dram_tensor("zr_scratch", [4, 1024], BF16).ap()

    def dram_ap(base, off, pattern):
        return bass.AP(tensor=base.tensor, offset=base.offset + off, ap=pattern)

    with tile.TileContext(nc) as tc:
        _build_tiled(nc, tc, x_d, wq_d, wqv_d, wo_d, b_d, out_d, zrc_d, dram_ap, zero_bias)
    nc.compile()
    return nc


def _build_tiled(nc, tc, x_d, wq_d, wqv_d, wo_d, b_d, out_d, zrc_d, dram_ap, zero_bias=False):
    from contextlib import ExitStack

    with ExitStack() as ctx:
        pers = ctx.enter_context(tc.tile_pool(name="pers", bufs=1))
        edp = ctx.enter_context(tc.tile_pool(name="edp", bufs=1))
        xwp = ctx.enter_context(tc.tile_pool(name="xw", bufs=1))
        wqvp = ctx.enter_context(tc.tile_pool(name="wqv", bufs=1))
        # dt PSUM pool spans phases A+B only; closed before phase C so its
        # 4 banks are free for oaug/y_ps (8-bank limit)
        dt_stack = ExitStack()
        psDT = dt_stack.enter_context(tc.tile_pool(name="psdt", bufs=1, space="PSUM"))

        # z-chain pools are bound at phase-B entry (zps lives in the B
        # PSUM pool; see z_chain)
        zpool = [None]

        # ---- persistent tiles ----
        # qkT[d-part, ot, n] : ot 0,1 = q dims 0..128,128..256; ot 2,3 = k
        qkT = pers.tile([128, 4, 1024], BF16)
        # v[m-part, mc, h, d] bf16
        v_sb = pers.tile([128, 8, 4, 64], BF16)
        # outT[i-part, ic, n-half] split per nh for fine-grained stage-3 deps
        outT = [pers.tile([128, 2, 512], BF16, name=f"outT{nh}") for nh in range(2)]
        # bias[c-part, r2, cg]
        bias_sb = pers.tile([128, 8, 4], F32)
        # ones column for the Z partition-reduce matmuls
        ones_sb = pers.tile([128, 4], BF16)
        nc.vector.memset(ones_sb[:], 1.0)

        # preload the exp activation table off the critical path
        et_in = pers.tile([64, 16], F32)
        et_out = pers.tile([64, 16], F32)
        nc.vector.memset(et_in[:], 0.0)
        nc.scalar.activation(
            et_out[:], et_in[:], mybir.ActivationFunctionType.Exp, scale=SCALE
        )

        # ---- attention logits + exp, buffered in SBUF until phase C ----
        # Alongside each block's exps, vector accumulates edsum = sum_mc ed
        # so Z (softmax denominator) is a tiny matmul + reciprocal chain
        # that completes DURING phase B — by the time attn@v runs, 1/Z is
        # already broadcast in SBUF and normalize is one vector multiply.
        eds = {}
        edsums = {}
        zbcs = {}

        def dots_exp(nh, hp, mc):
            dts = psDT.tile(
                [128, 2, 512], F32, tag="dt", bufs=2, name=f"dt_{nh}_{hp}_{mc}"
            )
            for h2 in range(2):
                b = h2 * 64
                nc.tensor.matmul(
                    dts[:, h2, :],
                    qkT[b : b + 64, 2 + hp, mc * 128 : (mc + 1) * 128],
                    qkT[b : b + 64, hp, nh * 512 : (nh + 1) * 512],
                    start=True,
                    stop=True,
                )
            ed = edp.tile(
                [128, 2, 512], BF16, tag="ed", bufs=32, name=f"ed_{nh}_{hp}_{mc}"
            )
            nc.scalar.activation(
                ed[:].rearrange("p a b -> p (a b)"),
                dts[:].rearrange("p a b -> p (a b)"),
                mybir.ActivationFunctionType.Exp,
                scale=SCALE,
            )
            eds[(nh, hp, mc)] = ed
            if mc == 0:
                # edsum lives in the wqv pool (room freed by ed's pool so
                # stage-3's y_t can triple-buffer)
                es = wqvp.tile(
                    [128, 2, 512], BF16, tag="edsum", bufs=4, name=f"es_{nh}_{hp}"
                )
                nc.vector.tensor_copy(es[:], ed[:])
                edsums[(nh, hp)] = es
            else:
                es = edsums[(nh, hp)]
                nc.vector.tensor_add(es[:], es[:], ed[:])
            if mc == 7:
                z_chain(nh, hp)

        def z_chain(nh, hp):
            # only ever called from phase B (mc==7 exps all live there), so
            # the znp pool and zpool[0] (B PSUM pool) exist by the first call
            slot = nh * 2 + hp
            eA = (nc.sync, nc.gpsimd)[slot % 2]
            es = edsums[(nh, hp)]
            zps = zpool[0].tile(
                [1, 2, 512], F32, tag="zps", bufs=1, name=f"zps_{slot}"
            )
            for h2 in range(2):
                nc.tensor.matmul(
                    zps[:, h2, :], ones_sb[:, 0:1], es[:, h2, :],
                    start=True, stop=True,
                )
            # small vector copy off PSUM so the zps bank (and the B PSUM
            # pool at close) frees without waiting a DMA round trip
            zsb = znp.tile([1, 2, 512], F32, tag="zsb", bufs=1, name=f"zsb_{slot}")
            nc.vector.tensor_copy(zsb[:], zps[:])
            z64 = znp.tile([64, 16], F32, tag="z64", bufs=2)
            eA.dma_start(out=z64[:], in_=zsb[0:1, :, :])
            z64r = znp.tile([64, 16], F32, tag="z64r", bufs=2)
            nc.vector.reciprocal(z64r[:], z64[:])
            # bf16 for the DRAM round trip + broadcast tile (1/Z at 0.4%
            # rounding is far inside the error budget; halves the SBUF and
            # broadcast-DMA cost)
            z64rb = znp.tile([64, 16], BF16, tag="z64rb", bufs=2)
            nc.vector.tensor_copy(z64rb[:], z64r[:])
            eA.dma_start(
                out=zrc_d[slot, :].rearrange("(a b) -> a b", a=64),
                in_=z64rb[:],
            )
            # 1/Z broadcast: partitions 0-63 get the h2=0 row, 64-127 the
            # h2=1 row, matching the col-tiled av output layout
            zbc = znp.tile([128, 512], BF16, tag="zbc", bufs=4, name=f"zbc_{slot}")
            eA.dma_start(
                out=zbc[0:64, :],
                in_=dram_ap(zrc_d, slot * 1024, [[0, 64], [1, 512]]),
            )
            eA.dma_start(
                out=zbc[64:128, :],
                in_=dram_ap(zrc_d, slot * 1024 + 512, [[0, 64], [1, 512]]),
            )
            zbcs[(nh, hp)] = zbc

        # ---- x tiles: w0 tiles as two r2-half tiles (4KB rows keep the
        # DMA queues at full rate; tile-granular deps gate only 512KB),
        # w1 tiles whole.  xtbs[(w,cg)] is a list of 8 [128,16,32] views.
        xtbs = {}

        def load_x_quarter(w, cg, quarter, eng):
            # only for the startup-critical first tile: 256KB granularity
            # lets the first matmul fire ~2us earlier
            t = xwp.tile(
                [128, 2, 16, 32], BF16, tag="xtb0q", bufs=4,
                name=f"xq_{w}_{cg}_{quarter}",
            )
            eng.dma_start(
                out=t[:],
                in_=dram_ap(
                    x_d,
                    (w * 4 + cg) * 128 * 4096 + quarter * 1024,
                    [[4096, 128], [1, 1024]],
                ),
            )
            xtbs.setdefault((w, cg), [None] * 8)
            for r2l in range(2):
                xtbs[(w, cg)][quarter * 2 + r2l] = t[:, r2l]

        def load_x_half(w, cg, half, eng):
            # w0 halves and w1 halves use separate tags: a shared tag would
            # make a w1-half DMA wait on a w0 buffer that only frees in
            # phase B — after qk-w1 already needs the data (deadlock)
            t = xwp.tile(
                [128, 4, 16, 32], BF16, tag=f"xtb{w}h", bufs=6 if w == 0 else 2,
                name=f"xh_{w}_{cg}_{half}",
            )
            eng.dma_start(
                out=t[:],
                in_=dram_ap(
                    x_d,
                    (w * 4 + cg) * 128 * 4096 + half * 2048,
                    [[4096, 128], [1, 2048]],
                ),
            )
            xtbs.setdefault((w, cg), [None] * 8)
            for r2l in range(4):
                xtbs[(w, cg)][half * 4 + r2l] = t[:, r2l]

        def load_x_whole(w, cg, eng):
            t = xwp.tile(
                [128, 8, 16, 32], BF16, tag="xtb1", bufs=3, name=f"xtb_{w}_{cg}"
            )
            eng.dma_start(
                out=t[:],
                in_=dram_ap(x_d, (w * 4 + cg) * 128 * 4096, [[4096, 128], [1, 4096]]),
            )
            xtbs[(w, cg)] = [t[:, r2] for r2 in range(8)]

        # wqv chunks live in their own pool (consumed in phase B)
        wqv_t = {}

        def load_wq_v(r2, i, eng):
            t = wqvp.tile([128, 2, 256], BF16, name=f"wqv_{r2}_{i}")
            eng.dma_start(
                out=t[:],
                in_=dram_ap(wqv_d, (r2 * 2 + i) * 65536, [[512, 128], [1, 512]]),
            )
            wqv_t[(r2, i)] = t

        # =========================== phase A ===========================
        # qk projection, both windows; dots+exp for (nh0, mc0-3)
        # interleaved into the w1 pass.
        with (
            tc.tile_pool(name="wq", bufs=1) as wqp,
            tc.tile_pool(name="ps1", bufs=1, space="PSUM") as ps1,
        ):
            wqk_t = {}

            def load_wq_qk(r2, i, eng):
                t = wqp.tile([128, 2, 512], BF16, name=f"wqk_{r2}_{i}")
                eng.dma_start(
                    out=t[:],
                    in_=dram_ap(wq_d, (r2 * 2 + i) * 131072, [[1024, 128], [1, 1024]]),
                )
                wqk_t[(r2, i)] = t

            # PE warmup: dummy matmuls so HAM is at 2.4 GHz when the first
            # real matmul arrives (~11us)
            warm = wqp.tile([128, 256], BF16)
            nc.vector.memset(warm[:], 0.0)
            warm_ps = ps1.tile([128, 512], F32, tag="qk0", bufs=1)
            for i in range(18):
                nc.tensor.matmul(
                    warm_ps[:, 0:256], warm[:, 0:128], warm[:], start=True, stop=True
                )

            # ---- DMA staging, consumption-ordered per queue (each queue
            # sustains only ~90-110 GB/s early with 8-core HBM contention;
            # every item below is placed so it lands just before the qk
            # matmul stream consumes it). ----
            # sync:   x00A w4 w5 x01A x01B x02A x02B x12 | wo[0] @B
            # scalar: wqk i0 r0-3, wqk i1 r0-3, x10, x13 | wqv r0-3 @mid-A
            # gpsimd: x00B w6 w7, i1 r4-7, x03A x03B x11 | wqv r4-7, bias
            load_x_quarter(0, 0, 0, nc.sync)
            load_wq_qk(0, 0, nc.scalar)
            load_x_quarter(0, 0, 2, nc.gpsimd)
            load_x_quarter(0, 0, 1, nc.sync)
            load_x_quarter(0, 0, 3, nc.gpsimd)
            load_wq_qk(1, 0, nc.scalar)
            load_wq_qk(4, 0, nc.sync)
            load_wq_qk(6, 0, nc.gpsimd)
            load_wq_qk(2, 0, nc.scalar)
            load_wq_qk(5, 0, nc.sync)
            load_wq_qk(7, 0, nc.gpsimd)
            load_wq_qk(3, 0, nc.scalar)
            for r2 in (0, 1, 2, 3):
                load_wq_qk(r2, 1, nc.scalar)
            for r2 in (4, 5, 6, 7):
                load_wq_qk(r2, 1, nc.gpsimd)
            load_x_half(0, 1, 0, nc.sync)
            load_x_half(0, 1, 1, nc.sync)
            load_x_half(0, 2, 0, nc.sync)
            load_x_half(0, 2, 1, nc.sync)
            load_x_half(0, 3, 0, nc.gpsimd)
            load_x_half(0, 3, 1, nc.gpsimd)
            load_x_half(1, 0, 0, nc.scalar)
            load_x_half(1, 0, 1, nc.scalar)
            load_x_whole(1, 2, nc.sync)
            load_x_whole(1, 1, nc.gpsimd)
            load_x_whole(1, 3, nc.scalar)

            # ---- qk matmul passes ----
            # per-cg r2 consumption order matched to DMA arrival order
            W0_ORDERS = [
                [0, 1, 6, 4, 2, 7, 5, 3],
                [0, 4, 1, 5, 2, 6, 3, 7],
                [0, 4, 1, 5, 2, 6, 3, 7],
                [0, 4, 1, 5, 2, 6, 3, 7],
            ]
            W1_ORDERS = [list(range(8))] * 4

            def qk_window(w, dots_list):
                # dots_list: (nh, hp, mc) tuples to interleave, 2 per cg
                orders = W0_ORDERS if w == 0 else W1_ORDERS
                qks = [
                    ps1.tile([128, 512], F32, tag=f"qk{ot}", bufs=1, name=f"qk_{w}_{ot}")
                    for ot in range(4)
                ]
                for cg in range(4):
                    xv = xtbs[(w, cg)]
                    for ri, r2 in enumerate(orders[cg]):
                        first = cg == 0 and ri == 0
                        last = cg == 3 and ri == 7
                        for ot in range(4):
                            nc.tensor.matmul(
                                qks[ot][:],
                                wqk_t[(r2, cg // 2)][
                                    :, cg % 2, ot * 128 : (ot + 1) * 128
                                ],
                                xv[r2][:],
                                start=first,
                                stop=last,
                            )
                    for d in dots_list[2 * cg : 2 * cg + 2]:
                        dots_exp(*d)
                # k evacuations (ot 2,3) first: earliest dots need them.
                # each split across scalar+vector so the w1 pass (reusing
                # the qk banks) unblocks in ~0.5us instead of ~1.5us
                for ot in (2, 3, 0, 1):
                    dst = qkT[:, ot, w * 512 : (w + 1) * 512]
                    nc.scalar.copy(dst[0:64], qks[ot][0:64, :])
                    nc.vector.tensor_copy(dst[64:128], qks[ot][64:128, :])

            qk_window(0, [])
            # wqv + bias issues between the windows: by now each engine's
            # ring has drained, and their data is needed only in phase B
            for i in range(2):
                for r2 in (0, 1, 2, 3):
                    load_wq_v(r2, i, nc.scalar)
                for r2 in (4, 5, 6, 7):
                    load_wq_v(r2, i, nc.gpsimd)
            # host pre-arranges b_out as [p, r2, cg] so this is a flat copy
            nc.gpsimd.dma_start(
                out=bias_sb[:],
                in_=dram_ap(b_d, 0, [[32, 128], [4, 8], [1, 4]]),
            )
            qk_window(1, [(0, 0, 0), (0, 0, 1), (0, 0, 2), (0, 0, 3),
                          (0, 1, 0), (0, 1, 1), (0, 1, 2), (0, 1, 3)])

        # =========================== phase B ===========================
        # v projection, both windows, with the remaining 24 dots+exp
        # pairs paced in (~1 per 11 v-matmuls ~= 1.2us of PE work).
        # W_out loads here (deadline: phase C), in the SBUF slot wqk freed.
        wop = ctx.enter_context(tc.tile_pool(name="wop", bufs=1))
        # z-chain tiles live B..C
        znp = ctx.enter_context(tc.tile_pool(name="znp", bufs=1))
        wo_sb = wop.tile([128, 2, 4096], BF16)  # [i-part, ic, c_perm]
        nc.sync.dma_start(
            out=wo_sb[:, 0, :],
            in_=dram_ap(wo_d, 0, [[4096, 128], [1, 4096]]),
        )
        nc.gpsimd.dma_start(
            out=wo_sb[:, 1, :],
            in_=dram_ap(wo_d, 524288, [[4096, 128], [1, 4096]]),
        )
        rem = (
            [(0, 0, mc) for mc in range(4, 8)]
            + [(0, 1, mc) for mc in range(4, 8)]
            + [(1, 0, mc) for mc in range(8)]
            + [(1, 1, mc) for mc in range(8)]
        )
        with tc.tile_pool(name="psv", bufs=1, space="PSUM") as psV:
            zpool[0] = psV
            mm_ctr = [0]
            di = [0]

            def maybe_dots():
                # insert the next dots pair every 11 v-matmuls
                if di[0] < len(rem) and mm_ctr[0] >= 11:
                    mm_ctr[0] = 0
                    dots_exp(*rem[di[0]])
                    di[0] += 1

            # s-outer: one token-chunk accumulation group at a time, so the
            # v pass needs only 2 PSUM banks (zps + dt take the rest)
            for w in range(2):
                for s in range(4):
                    vps = psV.tile(
                        [128, 256], F32, tag=f"v{s % 2}", bufs=1, name=f"v_{w}_{s}"
                    )
                    for cg in range(4):
                        xv = xtbs[(w, cg)]
                        for r2 in range(8):
                            nc.tensor.matmul(
                                vps[:],
                                xv[r2][:, 4 * s : 4 * s + 4, :],
                                wqv_t[(r2, cg // 2)][:, cg % 2, :],
                                start=(cg == 0 and r2 == 0),
                                stop=(cg == 3 and r2 == 7),
                            )
                            mm_ctr[0] += 1
                            maybe_dots()
                    nc.vector.tensor_copy(
                        v_sb[:, 4 * w + s, :, :],
                        vps[:].rearrange("p (h d) -> p h d", h=4),
                    )
            # flush any unpaced dots pairs (shouldn't happen: 256 MMs/11 > 23)
            while di[0] < len(rem):
                dots_exp(*rem[di[0]])
                di[0] += 1
            # keep-alive matmuls: the B->C pool-close barrier idles the PE
            # ~3-4us (last exp gates the dt release); these run through it
            # so HAM stays at 2.4 GHz into the av burst
            ka = psV.tile([128, 256], F32, tag="v0", bufs=1, name="ka")
            for i in range(16):
                nc.tensor.matmul(
                    ka[0:1, :], ones_sb[:, 0:1], qkT[:, 0, 0:256],
                    start=True, stop=True,
                )

        # dt banks free for phase C once the last exp has read them
        dt_stack.close()

        # =========================== phase C ===========================
        # av bursts + per-block normalize + output projection.
        av_stack = ExitStack()
        psAV = av_stack.enter_context(tc.tile_pool(name="psav", bufs=1, space="PSUM"))
        ps3 = [None]
        with tc.tile_pool(name="s3b", bufs=1) as s3:
            def av_block(nh, hp):
                # col-tiled av: h2=0 writes PSUM partitions 0-63, h2=1
                # writes 64-127 — disjoint col groups run concurrently, so
                # one (mc) step costs one N=512 matmul slot, and the output
                # lands directly in outT's partition layout
                oaug = psAV.tile(
                    [128, 512], F32, tag="oa", bufs=2, name=f"oaug_{nh}_{hp}"
                )
                for mc in range(8):
                    edc = eds[(nh, hp, mc)]
                    for h2 in range(2):
                        nc.tensor.matmul(
                            oaug[64 * h2 : 64 * h2 + 64, :],
                            v_sb[:, mc, 2 * hp + h2, :],
                            edc[:, h2, :],
                            start=(mc == 0),
                            stop=(mc == 7),
                        )
                # normalize: 1/Z was pre-broadcast during phase B — one mul
                nc.vector.tensor_mul(
                    outT[nh][:, hp, :], oaug[:], zbcs[(nh, hp)][:]
                )

            s3_idx = [0]

            def stage3_group(nqp, ct, half):
                # two nq tiles per group share every W_out LDWEIGHTS (one
                # load feeds two N=256 matmuls, fully hiding the weight-
                # load tax); 2 groups rotate through the 8 PSUM banks
                rot = s3_idx[0] % 2
                s3_idx[0] += 1
                nqs = (2 * nqp, 2 * nqp + 1)
                y_ps = {
                    nq: ps3[0].tile(
                        [128, 4, 256], F32, tag=f"yps{rot}{nq % 2}", bufs=1,
                        name=f"yps_{nq}_{ct}_{half}",
                    )
                    for nq in nqs
                }
                for r2l in range(4):
                    r2 = half * 4 + r2l
                    for ic in range(2):
                        for nq in nqs:
                            nc.tensor.matmul(
                                y_ps[nq][:, r2l, :],
                                wo_sb[
                                    :,
                                    ic,
                                    r2 * 512 + ct * 128 : r2 * 512 + (ct + 1) * 128,
                                ],
                                outT[nq // 2][
                                    :, ic, (nq % 2) * 256 : (nq % 2 + 1) * 256
                                ],
                                start=(r2l % 2 == 0 and ic == 0),
                                stop=(r2l % 2 == 1 and ic == 1),
                            )
                for nq in nqs:
                    y_t = s3.tile(
                        [128, 4, 256], BF16, tag=f"yt{nq % 2}", bufs=3,
                        name=f"yt_{nq}_{ct}_{half}",
                    )
                    if zero_bias:
                        # evacuation split across vector+scalar (~0.7us
                        # each < the ~0.9us of matmul work per tile);
                        # gpsimd has no PSUM port
                        nc.vector.tensor_copy(y_t[:, 0:2, :], y_ps[nq][:, 0:2, :])
                        nc.scalar.copy(y_t[:, 2:4, :], y_ps[nq][:, 2:4, :])
                    else:
                        bias_bc4 = bias_sb[
                            :, half * 4 : (half + 1) * 4, ct
                        ][:, :, None].broadcast_to([128, 4, 256])
                        nc.vector.tensor_add(
                            y_t[:, 0:2, :], y_ps[nq][:, 0:2, :], bias_bc4[:, 0:2, :]
                        )
                        nc.vector.tensor_add(
                            y_t[:, 2:4, :], y_ps[nq][:, 2:4, :], bias_bc4[:, 2:4, :]
                        )
                    # every tile drains as two 128KB DMAs (partition-split
                    # keeps 2KB contiguous rows) on sync/gpsimd only —
                    # scalar's evacuation stream must never stall on a
                    # ring-full DMA issue
                    for qq in range(2):
                        deng = (nc.sync, nc.gpsimd)[(nq + qq) % 2]
                        deng.dma_start(
                            out=dram_ap(
                                out_d,
                                ((nq * 4 + ct) * 2 + half) * 128 * 1024
                                + qq * 64 * 1024,
                                [[1024, 64], [1, 1024]],
                            ),
                            in_=y_t[64 * qq : 64 * qq + 64, :, :],
                        )

            av_block(0, 0)
            av_block(0, 1)
            av_block(1, 0)
            av_block(1, 1)
            # av PSUM banks handed over to the output-projection rotation
            av_stack.close()
            with tc.tile_pool(name="ps3", bufs=1, space="PSUM") as ps3p:
                ps3[0] = ps3p
                for nqp in range(2):
                    for ct in range(4):
                        for half in range(2):
                            stage3_group(nqp, ct, half)


def _get_nc(zero_bias=False):
    key = f"nc_zb{int(zero_bias)}"
    if key not in _CACHE:
        _CACHE[key] = _build(zero_bias=zero_bias)
    return _CACHE[key]


def _prep_weights(W_qkv, W_out, b_out):
    wq_perm = (
        W_qkv.reshape(64, 8, 8, 768).transpose(2, 0, 1, 3).reshape(4096, 768)
    )
    # split qk vs v columns and pack each in exact SBUF chunk layout
    # [r2, i(cg pair), p, (cgl, cols)] so every device load is one fully
    # contiguous DMA: rows within an (r2, i) chunk are (cgl*128 + p)
    def pack(cols):
        n = cols.shape[1]
        t = cols.reshape(8, 2, 2, 128, n)       # [r2, i, cgl, p, o]
        t = t.transpose(0, 1, 3, 2, 4)          # [r2, i, p, cgl, o]
        return np.ascontiguousarray(
            t.reshape(8, 2, 128, 2 * n)
        ).astype(ml_dtypes.bfloat16)

    wq_qk = pack(wq_perm[:, 0:512])
    wq_v = pack(wq_perm[:, 512:768])
    wo_perm = np.ascontiguousarray(
        W_out.reshape(256, 64, 8, 8).transpose(0, 3, 1, 2).reshape(256, 4096)
    ).astype(ml_dtypes.bfloat16)
    # b_perm[r2*512 + c0*8 + r1] = b_out[c0*64 + r1*8 + r2], then laid out
    # [p, r2, cg] where p = (c0 % 16)*8 + r1, cg = c0 // 16
    b_perm = b_out.reshape(64, 8, 8).transpose(2, 0, 1).reshape(4096)
    b_perm = np.ascontiguousarray(
        b_perm.reshape(8, 4, 128).transpose(2, 0, 1).reshape(4096)
    ).astype(np.float32)
    return wq_qk, wq_v, wo_perm, b_perm


def _pack_x(xb):
    # xb [64, 256, 256] f32 -> [w*4+cg, p=(c0%16)*8+r1, r2*512+hh*32+ww] bf16
    # x[c0, (w*16+hh)*8 + r1, ww*8 + r2]
    t = xb.astype(ml_dtypes.bfloat16)
    t = t.reshape(4, 16, 2, 16, 8, 32, 8)  # [cg, c0l, w, hh, r1, ww, r2]
    t = t.transpose(2, 0, 1, 4, 6, 3, 5)   # [w, cg, c0l, r1, r2, hh, ww]
    return np.ascontiguousarray(t.reshape(8, 128, 4096))


def _unpack_out(raw):
    # raw [32, 128, 1024] = [(nq*4+ct)*2+half, (c0%16)*8+r1, r2l*256+hq*32+ww]
    # with r2 = half*4 + r2l -> y[c0, (nq*8+hq)*8 + r1, ww*8 + r2]
    t = np.asarray(raw).reshape(4, 4, 2, 16, 8, 4, 8, 32)
    # [nq, ct, half, c0l, r1, r2l, hq, ww]
    t = t.transpose(1, 3, 0, 6, 4, 7, 2, 5)  # [ct, c0l, nq, hq, r1, ww, half, r2l]
    return t.reshape(64, 256, 256)


def kernel(x, W_qkv, W_out, b_out):
    nc = _get_nc(zero_bias=not np.any(np.asarray(b_out)))
    wq_qk, wq_v, wo_perm, b_perm = _prep_weights(
        np.asarray(W_qkv, dtype=np.float32),
        np.asarray(W_out, dtype=np.float32),
        np.asarray(b_out, dtype=np.float32),
    )

    in_maps = [
        {
            "x": _pack_x(np.asarray(x[b], dtype=np.float32)),
            "W_qkv": wq_qk,
            "W_qkv_v": wq_v,
            "W_out": wo_perm,
            "b_out": b_perm,
        }
        for b in range(8)
    ]
    trace = bool(int(os.environ.get("BENCH_TRACE", "0")))
    if trace:
        try:  # tracing needs the NTFF hook shim (see test.py); degrade if absent
            from antenv.axon_hooks import get_axon_ntff_profile_hook  # noqa: F401
        except ImportError:
            trace = False
    res = run_bass_kernel_spmd(nc, in_maps, core_ids=list(range(8)), trace=trace)
    if trace:
        _CACHE["last_result"] = res
    return np.stack(
        [_unpack_out(res.results[b]["out"]) for b in range(8)]
    ).astype(np.float32)


# revision 50
# speedup vs baseline: 1.0397x; 1.0397x over previous
"""Trainium2 Bass kernel for PixelUnshuffle->MHA->PixelShuffle (nn_Attention).

Reference computation (per batch element, 8 batch elements data-parallel
across 8 NeuronCores):
  x [64, 256, 256] --PixelUnshuffle(8)--> tokens [N=1024, C=4096]
  qkv = tokens @ W_qkv            [1024, 768]
  4-head attention (d=64), softmax over tokens
  y = attn_out @ W_out + b_out    [1024, 4096]
  --PixelShuffle(8)--> [64, 256, 256]

v4 structure: the exp-bound attention stage is dissolved into the
projection stages so the scalar engine's exp stream (the old stage-2
floor, ~37us) hides under PE matmul work:

  Phase A: qk projections for both token windows (PE-dense).  The
    window-0 attention logits (dotsT = kT x qT) + exps for n-half 0,
    m-chunks 0-3 are interleaved into the window-1 qk pass.
  Phase B: v projections for both windows with ALL remaining dots+exp
    pairs interleaved (paced ~1 pair per 1.2us of PE work so the
    in-order PE queue never stalls on the dt PSUM double-buffer).  The
    exp outputs (ed tiles, bf16) are buffered in SBUF (64KB/partition).
  Phase C: attn@v runs as a dense PE burst per (n-half, head-pair)
    block (no exp dependency left), each block's 1/Z normalize chain
    (SBUF redistribute DMA + reciprocal + DRAM-round-trip broadcast)
    overlaps later blocks, and the output projection starts as soon as
    outT[0] is normalized.

Attention math is unchanged from v3: dotsT[m, n] summed-token-on-
partitions, exp on scalar (the only exp engine), a ones column in v
accumulating the softmax denominator Z for free (row 64 of oaug).

Startup: the first x tile is split into 8 per-r2 SLICE TILES (tile
dependency tracking is tile-granular) so the first real matmul needs
only 128KB of x + one 256KB weight chunk (~11us) instead of the whole
1MB tile.  All queues are loaded in consumption order; aggregate early
HBM bandwidth is ~300 GB/s (8-core contention), which bounds the qk-w0
pass; the r2 consumption order [0,4,1,5,2,6,3,7] alternates the
scalar/gpsimd weight queues to match arrival.

Layout/host packing identical to v3: x pre-packed+pre-cast bf16 into
[w*4+cg, p, r2*512 + hh*32 + ww] tiles; W_qkv split qk/v columns in
exact SBUF chunk layout; output written as raw [nq, ct, p, hq, ww, r2]
bf16 tiles and pixel-shuffled + upcast on the host.
"""

import sys

if "/opt/trn_rl_repo" not in sys.path:
    sys.path.insert(0, "/opt/trn_rl_repo")

import os

import ml_dtypes
import numpy as np

import concourse.bass as bass
from concourse import bacc, mybir, tile
from concourse.bass_utils import run_bass_kernel_spmd

F32 = mybir.dt.float32
BF16 = mybir.dt.bfloat16

SCALE = 0.125  # DIM_HEAD ** -0.5

_CACHE = {}

R2_ORDER = [0, 4, 1, 5, 2, 6, 3, 7]


def _build(zero_bias=False):
    nc = bacc.Bacc("TRN2", target_bir_lowering=False, debug=False, num_devices=8)

    # x pre-packed on host: [w*4+cg, p, r2*512 + hh*32 + ww]
    x_d = nc.dram_tensor("x", [8, 128, 4096], BF16, kind="ExternalInput").ap()
    # W_qkv split on host into qk columns and v columns, each pre-packed in
    # exact SBUF chunk layout so every load is one fully contiguous DMA
    wq_d = nc.dram_tensor("W_qkv", [8, 2, 128, 1024], BF16, kind="ExternalInput").ap()
    wqv_d = nc.dram_tensor("W_qkv_v", [8, 2, 128, 512], BF16, kind="ExternalInput").ap()
    wo_d = nc.dram_tensor("W_out", [256, 4096], BF16, kind="ExternalInput").ap()
    b_d = nc.dram_tensor("b_out", [4096], F32, kind="ExternalInput").ap()
    # raw output tiles: [(nq*4+ct)*2+half, p, r2l*256 + hq*32 + ww] with
    # r2 = half*4 + r2l; host pixel-shuffles
    out_d = nc.dram_tensor("out", [32, 128, 1024], BF16, kind="ExternalOutput").ap()

    zrc_d = nc.dram_tensor("zr_scratch", [4, 1024], BF16).ap()

    def dram_ap(base, off, pattern):
        return bass.AP(tensor=base.tensor, offset=base.offset + off, ap=pattern)

    with tile.TileContext(nc) as tc:
        _build_tiled(nc, tc, x_d, wq_d, wqv_d, wo_d, b_d, out_d, zrc_d, dram_ap, zero_bias)
    nc.compile()
    return nc


def _build_tiled(nc, tc, x_d, wq_d, wqv_d, wo_d, b_d, out_d, zrc_d, dram_ap, zero_bias=False):
    from contextlib import ExitStack

    with ExitStack() as ctx:
        pers = ctx.enter_context(tc.tile_pool(name="pers", bufs=1))
        edp = ctx.enter_context(tc.tile_pool(name="edp", bufs=1))
        xwp = ctx.enter_context(tc.tile_pool(name="xw", bufs=1))
        wqvp = ctx.enter_context(tc.tile_pool(name="wqv", bufs=1))
        # dt PSUM pool spans phases A+B only; closed before phase C so its
        # 4 banks are free for oaug/y_ps (8-bank limit)
        dt_stack = ExitStack()
        psDT = dt_stack.enter_context(tc.tile_pool(name="psdt", bufs=1, space="PSUM"))

        # z-chain pools are bound at phase-B entry (zps lives in the B
        # PSUM pool; see z_chain)
        zpool = [None]

        # ---- persistent tiles ----
        # qkT[d-part, ot, n] : ot 0,1 = q dims 0..128,128..256; ot 2,3 = k
        qkT = pers.tile([128, 4, 1024], BF16)
        # v[m-part, mc, h, d] bf16
        v_sb = pers.tile([128, 8, 4, 64], BF16)
        # outT[i-part, ic, n-half] split per nh for fine-grained stage-3 deps
        outT = [pers.tile([128, 2, 512], BF16, name=f"outT{nh}") for nh in range(2)]
        # bias[c-part, r2, cg]
        bias_sb = pers.tile([128, 8, 4], F32)
        # ones column for the Z partition-reduce matmuls
        ones_sb = pers.tile([128, 4], BF16)
        nc.vector.memset(ones_sb[:], 1.0)

        # preload the exp activation table off the critical path
        et_in = pers.tile([64, 16], F32)
        et_out = pers.tile([64, 16], F32)
        nc.vector.memset(et_in[:], 0.0)
        nc.scalar.activation(
            et_out[:], et_in[:], mybir.ActivationFunctionType.Exp, scale=SCALE
        )

        # ---- attention logits + exp, buffered in SBUF until phase C ----
        # Alongside each block's exps, vector accumulates edsum = sum_mc ed
        # so Z (softmax denominator) is a tiny matmul + reciprocal chain
        # that completes DURING phase B — by the time attn@v runs, 1/Z is
        # already broadcast in SBUF and normalize is one vector multiply.
        eds = {}
        edsums = {}
        zbcs = {}

        def dots_exp(nh, hp, mc):
            dts = psDT.tile(
                [128, 2, 512], F32, tag="dt", bufs=2, name=f"dt_{nh}_{hp}_{mc}"
            )
            for h2 in range(2):
                b = h2 * 64
                nc.tensor.matmul(
                    dts[:, h2, :],
                    qkT[b : b + 64, 2 + hp, mc * 128 : (mc + 1) * 128],
                    qkT[b : b + 64, hp, nh * 512 : (nh + 1) * 512],
                    start=True,
                    stop=True,
                )
            ed = edp.tile(
                [128, 2, 512], BF16, tag="ed", bufs=32, name=f"ed_{nh}_{hp}_{mc}"
            )
            nc.scalar.activation(
                ed[:].rearrange("p a b -> p (a b)"),
                dts[:].rearrange("p a b -> p (a b)"),
                mybir.ActivationFunctionType.Exp,
                scale=SCALE,
            )
            eds[(nh, hp, mc)] = ed
            if mc == 0:
                # edsum lives in the wqv pool (room freed by ed's pool so
                # stage-3's y_t can triple-buffer)
                es = wqvp.tile(
                    [128, 2, 512], BF16, tag="edsum", bufs=4, name=f"es_{nh}_{hp}"
                )
                nc.vector.tensor_copy(es[:], ed[:])
                edsums[(nh, hp)] = es
            else:
                es = edsums[(nh, hp)]
                nc.vector.tensor_add(es[:], es[:], ed[:])
            if mc == 7:
                z_chain(nh, hp)

        def z_chain(nh, hp):
            # only ever called from phase B (mc==7 exps all live there), so
            # the znp pool and zpool[0] (B PSUM pool) exist by the first call
            slot = nh * 2 + hp
            eA = (nc.sync, nc.gpsimd)[slot % 2]
            es = edsums[(nh, hp)]
            zps = zpool[0].tile(
                [1, 2, 512], F32, tag="zps", bufs=1, name=f"zps_{slot}"
            )
            for h2 in range(2):
                nc.tensor.matmul(
                    zps[:, h2, :], ones_sb[:, 0:1], es[:, h2, :],
                    start=True, stop=True,
                )
            # small vector copy off PSUM so the zps bank (and the B PSUM
            # pool at close) frees without waiting a DMA round trip
            zsb = znp.tile([1, 2, 512], F32, tag="zsb", bufs=1, name=f"zsb_{slot}")
            nc.vector.tensor_copy(zsb[:], zps[:])
            z64 = znp.tile([64, 16], F32, tag="z64", bufs=2)
            eA.dma_start(out=z64[:], in_=zsb[0:1, :, :])
            z64r = znp.tile([64, 16], F32, tag="z64r", bufs=2)
            nc.vector.reciprocal(z64r[:], z64[:])
            # bf16 for the DRAM round trip + broadcast tile (1/Z at 0.4%
            # rounding is far inside the error budget; halves the SBUF and
            # broadcast-DMA cost)
            z64rb = znp.tile([64, 16], BF16, tag="z64rb", bufs=2)
            nc.vector.tensor_copy(z64rb[:], z64r[:])
            eA.dma_start(
                out=zrc_d[slot, :].rearrange("(a b) -> a b", a=64),
                in_=z64rb[:],
            )
            # 1/Z broadcast: partitions 0-63 get the h2=0 row, 64-127 the
            # h2=1 row, matching the col-tiled av output layout
            zbc = znp.tile([128, 512], BF16, tag="zbc", bufs=4, name=f"zbc_{slot}")
            eA.dma_start(
                out=zbc[0:64, :],
                in_=dram_ap(zrc_d, slot * 1024, [[0, 64], [1, 512]]),
            )
            eA.dma_start(
                out=zbc[64:128, :],
                in_=dram_ap(zrc_d, slot * 1024 + 512, [[0, 64], [1, 512]]),
            )
            zbcs[(nh, hp)] = zbc

        # ---- x tiles: w0 tiles as two r2-half tiles (4KB rows keep the
        # DMA queues at full rate; tile-granular deps gate only 512KB),
        # w1 tiles whole.  xtbs[(w,cg)] is a list of 8 [128,16,32] views.
        xtbs = {}

        def load_x_quarter(w, cg, quarter, eng):
            # only for the startup-critical first tile: 256KB granularity
            # lets the first matmul fire ~2us earlier
            t = xwp.tile(
                [128, 2, 16, 32], BF16, tag="xtb0q", bufs=4,
                name=f"xq_{w}_{cg}_{quarter}",
            )
            eng.dma_start(
                out=t[:],
                in_=dram_ap(
                    x_d,
                    (w * 4 + cg) * 128 * 4096 + quarter * 1024,
                    [[4096, 128], [1, 1024]],
                ),
            )
            xtbs.setdefault((w, cg), [None] * 8)
            for r2l in range(2):
                xtbs[(w, cg)][quarter * 2 + r2l] = t[:, r2l]

        def load_x_half(w, cg, half, eng):
            # w0 halves and w1 halves use separate tags: a shared tag would
            # make a w1-half DMA wait on a w0 buffer that only frees in
            # phase B — after qk-w1 already needs the data (deadlock)
            t = xwp.tile(
                [128, 4, 16, 32], BF16, tag=f"xtb{w}h", bufs=8 if w == 0 else 2,
                name=f"xh_{w}_{cg}_{half}",
            )
            eng.dma_start(
                out=t[:],
                in_=dram_ap(
                    x_d,
                    (w * 4 + cg) * 128 * 4096 + half * 2048,
                    [[4096, 128], [1, 2048]],
                ),
            )
            xtbs.setdefault((w, cg), [None] * 8)
            for r2l in range(4):
                xtbs[(w, cg)][half * 4 + r2l] = t[:, r2l]

        def load_x_whole(w, cg, eng):
            t = xwp.tile(
                [128, 8, 16, 32], BF16, tag="xtb1", bufs=3, name=f"xtb_{w}_{cg}"
            )
            eng.dma_start(
                out=t[:],
                in_=dram_ap(x_d, (w * 4 + cg) * 128 * 4096, [[4096, 128], [1, 4096]]),
            )
            xtbs[(w, cg)] = [t[:, r2] for r2 in range(8)]

        # wqv chunks live in their own pool (consumed in phase B)
        wqv_t = {}

        def load_wq_v(r2, i, eng):
            t = wqvp.tile([128, 2, 256], BF16, name=f"wqv_{r2}_{i}")
            eng.dma_start(
                out=t[:],
                in_=dram_ap(wqv_d, (r2 * 2 + i) * 65536, [[512, 128], [1, 512]]),
            )
            wqv_t[(r2, i)] = t

        # =========================== phase A ===========================
        # qk projection, both windows; dots+exp for (nh0, mc0-3)
        # interleaved into the w1 pass.
        with (
            tc.tile_pool(name="wq", bufs=1) as wqp,
            tc.tile_pool(name="ps1", bufs=1, space="PSUM") as ps1,
        ):
            wqk_t = {}

            def load_wq_qk(r2, i, eng):
                t = wqp.tile([128, 2, 512], BF16, name=f"wqk_{r2}_{i}")
                eng.dma_start(
                    out=t[:],
                    in_=dram_ap(wq_d, (r2 * 2 + i) * 131072, [[1024, 128], [1, 1024]]),
                )
                wqk_t[(r2, i)] = t

            # PE warmup: dummy matmuls so HAM is at 2.4 GHz when the first
            # real matmul arrives (~11us)
            warm = wqp.tile([128, 256], BF16)
            nc.vector.memset(warm[:], 0.0)
            warm_ps = ps1.tile([128, 512], F32, tag="qk0", bufs=1)
            for i in range(18):
                nc.tensor.matmul(
                    warm_ps[:, 0:256], warm[:, 0:128], warm[:], start=True, stop=True
                )

            # ---- DMA staging, consumption-ordered per queue (each queue
            # sustains only ~90-110 GB/s early with 8-core HBM contention;
            # every item below is placed so it lands just before the qk
            # matmul stream consumes it). ----
            # sync:   x00A w4 w5 x01A x01B x02A x02B x12 | wo[0] @B
            # scalar: wqk i0 r0-3, wqk i1 r0-3, x10, x13 | wqv r0-3 @mid-A
            # gpsimd: x00B w6 w7, i1 r4-7, x03A x03B x11 | wqv r4-7, bias
            load_x_half(0, 0, 0, nc.sync)
            load_wq_qk(0, 0, nc.scalar)
            load_x_half(0, 0, 1, nc.gpsimd)
            load_wq_qk(1, 0, nc.scalar)
            load_wq_qk(4, 0, nc.sync)
            load_wq_qk(6, 0, nc.gpsimd)
            load_wq_qk(2, 0, nc.scalar)
            load_wq_qk(5, 0, nc.sync)
            load_wq_qk(7, 0, nc.gpsimd)
            load_wq_qk(3, 0, nc.scalar)
            for r2 in (0, 1, 2, 3):
                load_wq_qk(r2, 1, nc.scalar)
            for r2 in (4, 5, 6, 7):
                load_wq_qk(r2, 1, nc.gpsimd)
            load_x_half(0, 1, 0, nc.sync)
            load_x_half(0, 1, 1, nc.sync)
            load_x_half(0, 2, 0, nc.sync)
            load_x_half(0, 2, 1, nc.sync)
            load_x_half(0, 3, 0, nc.gpsimd)
            load_x_half(0, 3, 1, nc.gpsimd)
            load_x_half(1, 0, 0, nc.scalar)
            load_x_half(1, 0, 1, nc.scalar)
            load_x_whole(1, 2, nc.sync)
            load_x_whole(1, 1, nc.gpsimd)
            load_x_whole(1, 3, nc.scalar)

            # ---- qk matmul passes ----
            # per-cg r2 consumption order matched to DMA arrival order
            W0_ORDERS = [
                [0, 1, 6, 4, 2, 7, 5, 3],
                [0, 4, 1, 5, 2, 6, 3, 7],
                [0, 4, 1, 5, 2, 6, 3, 7],
                [0, 4, 1, 5, 2, 6, 3, 7],
            ]
            W1_ORDERS = [list(range(8))] * 4

            def qk_window(w, dots_list):
                # dots_list: (nh, hp, mc) tuples to interleave, 2 per cg
                orders = W0_ORDERS if w == 0 else W1_ORDERS
                qks = [
                    ps1.tile([128, 512], F32, tag=f"qk{ot}", bufs=1, name=f"qk_{w}_{ot}")
                    for ot in range(4)
                ]
                for cg in range(4):
                    xv = xtbs[(w, cg)]
                    for ri, r2 in enumerate(orders[cg]):
                        first = cg == 0 and ri == 0
                        last = cg == 3 and ri == 7
                        for ot in range(4):
                            nc.tensor.matmul(
                                qks[ot][:],
                                wqk_t[(r2, cg // 2)][
                                    :, cg % 2, ot * 128 : (ot + 1) * 128
                                ],
                                xv[r2][:],
                                start=first,
                                stop=last,
                            )
                    for d in dots_list[2 * cg : 2 * cg + 2]:
                        dots_exp(*d)
                # k evacuations (ot 2,3) first: earliest dots need them
                for ot in (2, 3, 0, 1):
                    dst = qkT[:, ot, w * 512 : (w + 1) * 512]
                    if ot % 2 == 0:
                        nc.scalar.copy(dst, qks[ot][:])
                    else:
                        nc.vector.tensor_copy(dst, qks[ot][:])

            qk_window(0, [])
            # wqv + bias issues between the windows: by now each engine's
            # ring has drained, and their data is needed only in phase B
            for i in range(2):
                for r2 in (0, 1, 2, 3):
                    load_wq_v(r2, i, nc.scalar)
                for r2 in (4, 5, 6, 7):
                    load_wq_v(r2, i, nc.gpsimd)
            # host pre-arranges b_out as [p, r2, cg] so this is a flat copy
            nc.gpsimd.dma_start(
                out=bias_sb[:],
                in_=dram_ap(b_d, 0, [[32, 128], [4, 8], [1, 4]]),
            )
            qk_window(1, [(0, 0, 0), (0, 0, 1), (0, 0, 2), (0, 0, 3),
                          (0, 1, 0), (0, 1, 1), (0, 1, 2), (0, 1, 3)])

        # =========================== phase B ===========================
        # v projection, both windows, with the remaining 24 dots+exp
        # pairs paced in (~1 per 11 v-matmuls ~= 1.2us of PE work).
        # W_out loads here (deadline: phase C), in the SBUF slot wqk freed.
        wop = ctx.enter_context(tc.tile_pool(name="wop", bufs=1))
        # z-chain tiles live B..C
        znp = ctx.enter_context(tc.tile_pool(name="znp", bufs=1))
        wo_sb = wop.tile([128, 2, 4096], BF16)  # [i-part, ic, c_perm]
        nc.sync.dma_start(
            out=wo_sb[:, 0, :],
            in_=dram_ap(wo_d, 0, [[4096, 128], [1, 4096]]),
        )
        nc.gpsimd.dma_start(
            out=wo_sb[:, 1, :],
            in_=dram_ap(wo_d, 524288, [[4096, 128], [1, 4096]]),
        )
        rem = (
            [(0, 0, mc) for mc in range(4, 8)]
            + [(0, 1, mc) for mc in range(4, 8)]
            + [(1, 0, mc) for mc in range(8)]
            + [(1, 1, mc) for mc in range(8)]
        )
        with tc.tile_pool(name="psv", bufs=1, space="PSUM") as psV:
            zpool[0] = psV
            mm_ctr = [0]
            di = [0]

            def maybe_dots():
                # insert the next dots pair every 11 v-matmuls
                if di[0] < len(rem) and mm_ctr[0] >= 11:
                    mm_ctr[0] = 0
                    dots_exp(*rem[di[0]])
                    di[0] += 1

            # s-outer: one token-chunk accumulation group at a time, so the
            # v pass needs only 2 PSUM banks (zps + dt take the rest)
            for w in range(2):
                for s in range(4):
                    vps = psV.tile(
                        [128, 256], F32, tag=f"v{s % 2}", bufs=1, name=f"v_{w}_{s}"
                    )
                    for cg in range(4):
                        xv = xtbs[(w, cg)]
                        for r2 in range(8):
                            nc.tensor.matmul(
                                vps[:],
                                xv[r2][:, 4 * s : 4 * s + 4, :],
                                wqv_t[(r2, cg // 2)][:, cg % 2, :],
                                start=(cg == 0 and r2 == 0),
                                stop=(cg == 3 and r2 == 7),
                            )
                            mm_ctr[0] += 1
                            maybe_dots()
                    nc.vector.tensor_copy(
                        v_sb[:, 4 * w + s, :, :],
                        vps[:].rearrange("p (h d) -> p h d", h=4),
                    )
            # flush any unpaced dots pairs (shouldn't happen: 256 MMs/11 > 23)
            while di[0] < len(rem):
                dots_exp(*rem[di[0]])
                di[0] += 1
            # keep-alive matmuls: the B->C pool-close barrier idles the PE
            # ~3-4us (last exp gates the dt release); these run through it
            # so HAM stays at 2.4 GHz into the av burst
            ka = psV.tile([128, 256], F32, tag="v0", bufs=1, name="ka")
            for i in range(16):
                nc.tensor.matmul(
                    ka[0:1, :], ones_sb[:, 0:1], qkT[:, 0, 0:256],
                    start=True, stop=True,
                )

        # dt banks free for phase C once the last exp has read them
        dt_stack.close()

        # =========================== phase C ===========================
        # av bursts + per-block normalize + output projection.
        av_stack = ExitStack()
        psAV = av_stack.enter_context(tc.tile_pool(name="psav", bufs=1, space="PSUM"))
        ps3 = [None]
        with tc.tile_pool(name="s3b", bufs=1) as s3:
            def av_block(nh, hp):
                # col-tiled av: h2=0 writes PSUM partitions 0-63, h2=1
                # writes 64-127 — disjoint col groups run concurrently, so
                # one (mc) step costs one N=512 matmul slot, and the output
                # lands directly in outT's partition layout
                oaug = psAV.tile(
                    [128, 512], F32, tag="oa", bufs=2, name=f"oaug_{nh}_{hp}"
                )
                for mc in range(8):
                    edc = eds[(nh, hp, mc)]
                    for h2 in range(2):
                        nc.tensor.matmul(
                            oaug[64 * h2 : 64 * h2 + 64, :],
                            v_sb[:, mc, 2 * hp + h2, :],
                            edc[:, h2, :],
                            start=(mc == 0),
                            stop=(mc == 7),
                        )
                # normalize: 1/Z was pre-broadcast during phase B — one mul
                nc.vector.tensor_mul(
                    outT[nh][:, hp, :], oaug[:], zbcs[(nh, hp)][:]
                )

            s3_idx = [0]

            def stage3_group(nqp, ct, half):
                # two nq tiles per group share every W_out LDWEIGHTS (one
                # load feeds two N=256 matmuls, fully hiding the weight-
                # load tax); 2 groups rotate through the 8 PSUM banks
                rot = s3_idx[0] % 2
                s3_idx[0] += 1
                nqs = (2 * nqp, 2 * nqp + 1)
                y_ps = {
                    nq: ps3[0].tile(
                        [128, 4, 256], F32, tag=f"yps{rot}{nq % 2}", bufs=1,
                        name=f"yps_{nq}_{ct}_{half}",
                    )
                    for nq in nqs
                }
                for r2l in range(4):
                    r2 = half * 4 + r2l
                    for ic in range(2):
                        for nq in nqs:
                            nc.tensor.matmul(
                                y_ps[nq][:, r2l, :],
                                wo_sb[
                                    :,
                                    ic,
                                    r2 * 512 + ct * 128 : r2 * 512 + (ct + 1) * 128,
                                ],
                                outT[nq // 2][
                                    :, ic, (nq % 2) * 256 : (nq % 2 + 1) * 256
                                ],
                                start=(r2l % 2 == 0 and ic == 0),
                                stop=(r2l % 2 == 1 and ic == 1),
                            )
                for nq in nqs:
                    y_t = s3.tile(
                        [128, 4, 256], BF16, tag=f"yt{nq % 2}", bufs=3,
                        name=f"yt_{nq}_{ct}_{half}",
                    )
                    if zero_bias:
                        # evacuation split across vector+scalar (~0.7us
                        # each < the ~0.9us of matmul work per tile);
                        # gpsimd has no PSUM port
                        nc.vector.tensor_copy(y_t[:, 0:2, :], y_ps[nq][:, 0:2, :])
                        nc.scalar.copy(y_t[:, 2:4, :], y_ps[nq][:, 2:4, :])
                    else:
                        bias_bc4 = bias_sb[
                            :, half * 4 : (half + 1) * 4, ct
                        ][:, :, None].broadcast_to([128, 4, 256])
                        nc.vector.tensor_add(
                            y_t[:, 0:2, :], y_ps[nq][:, 0:2, :], bias_bc4[:, 0:2, :]
                        )
                        nc.vector.tensor_add(
                            y_t[:, 2:4, :], y_ps[nq][:, 2:4, :], bias_bc4[:, 2:4, :]
                        )
                    # every tile drains as two 128KB DMAs (partition-split
                    # keeps 2KB contiguous rows) on sync/gpsimd only —
                    # scalar's evacuation stream must never stall on a
                    # ring-full DMA issue
                    for qq in range(2):
                        deng = (nc.sync, nc.gpsimd)[(nq + qq) % 2]
                        deng.dma_start(
                            out=dram_ap(
                                out_d,
                                ((nq * 4 + ct) * 2 + half) * 128 * 1024
                                + qq * 64 * 1024,
                                [[1024, 64], [1, 1024]],
                            ),
                            in_=y_t[64 * qq : 64 * qq + 64, :, :],
                        )

            av_block(0, 0)
            av_block(0, 1)
            av_block(1, 0)
            av_block(1, 1)
            # av PSUM banks handed over to the output-projection rotation
            av_stack.close()
            with tc.tile_pool(name="ps3", bufs=1, space="PSUM") as ps3p:
                ps3[0] = ps3p
                for nqp in range(2):
                    for ct in range(4):
                        for half in range(2):
                            stage3_group(nqp, ct, half)


def _get_nc(zero_bias=False):
    key = f"nc_zb{int(zero_bias)}"
    if key not in _CACHE:
        _CACHE[key] = _build(zero_bias=zero_bias)
    return _CACHE[key]


def _prep_weights(W_qkv, W_out, b_out):
    wq_perm = (
        W_qkv.reshape(64, 8, 8, 768).transpose(2, 0, 1, 3).reshape(4096, 768)
    )
    # split qk vs v columns and pack each in exact SBUF chunk layout
    # [r2, i(cg pair), p, (cgl, cols)] so every device load is one fully
    # contiguous DMA: rows within an (r2, i) chunk are (cgl*128 + p)
    def pack(cols):
        n = cols.shape[1]
        t = cols.reshape(8, 2, 2, 128, n)       # [r2, i, cgl, p, o]
        t = t.transpose(0, 1, 3, 2, 4)          # [r2, i, p, cgl, o]
        return np.ascontiguousarray(
            t.reshape(8, 2, 128, 2 * n)
        ).astype(ml_dtypes.bfloat16)

    wq_qk = pack(wq_perm[:, 0:512])
    wq_v = pack(wq_perm[:, 512:768])
    wo_perm = np.ascontiguousarray(
        W_out.reshape(256, 64, 8, 8).transpose(0, 3, 1, 2).reshape(256, 4096)
    ).astype(ml_dtypes.bfloat16)
    # b_perm[r2*512 + c0*8 + r1] = b_out[c0*64 + r1*8 + r2], then laid out
    # [p, r2, cg] where p = (c0 % 16)*8 + r1, cg = c0 // 16
    b_perm = b_out.reshape(64, 8, 8).transpose(2, 0, 1).reshape(4096)
    b_perm = np.ascontiguousarray(
        b_perm.reshape(8, 4, 128).transpose(2, 0, 1).reshape(4096)
    ).astype(np.float32)
    return wq_qk, wq_v, wo_perm, b_perm


def _pack_x(xb):
    # xb [64, 256, 256] f32 -> [w*4+cg, p=(c0%16)*8+r1, r2*512+hh*32+ww] bf16
    # x[c0, (w*16+hh)*8 + r1, ww*8 + r2]
    t = xb.astype(ml_dtypes.bfloat16)
    t = t.reshape(4, 16, 2, 16, 8, 32, 8)  # [cg, c0l, w, hh, r1, ww, r2]
    t = t.transpose(2, 0, 1, 4, 6, 3, 5)   # [w, cg, c0l, r1, r2, hh, ww]
    return np.ascontiguousarray(t.reshape(8, 128, 4096))


def _unpack_out(raw):
    # raw [32, 128, 1024] = [(nq*4+ct)*2+half, (c0%16)*8+r1, r2l*256+hq*32+ww]
    # with r2 = half*4 + r2l -> y[c0, (nq*8+hq)*8 + r1, ww*8 + r2]
    t = np.asarray(raw).reshape(4, 4, 2, 16, 8, 4, 8, 32)
    # [nq, ct, half, c0l, r1, r2l, hq, ww]
    t = t.transpose(1, 3, 0, 6, 4, 7, 2, 5)  # [ct, c0l, nq, hq, r1, ww, half, r2l]
    return t.reshape(64, 256, 256)


def kernel(x, W_qkv, W_out, b_out):
    nc = _get_nc(zero_bias=not np.any(np.asarray(b_out)))
    wq_qk, wq_v, wo_perm, b_perm = _prep_weights(
        np.asarray(W_qkv, dtype=np.float32),
        np.asarray(W_out, dtype=np.float32),
        np.asarray(b_out, dtype=np.float32),
    )

    in_maps = [
        {
            "x": _pack_x(np.asarray(x[b], dtype=np.float32)),
            "W_qkv": wq_qk,
            "W_qkv_v": wq_v,
            "W_out": wo_perm,
            "b_out": b_perm,
        }
        for b in range(8)
    ]
    trace = bool(int(os.environ.get("BENCH_TRACE", "0")))
    if trace:
        try:  # tracing needs the NTFF hook shim (see test.py); degrade if absent
            from antenv.axon_hooks import get_axon_ntff_profile_hook  # noqa: F401
        except ImportError:
            trace = False
    res = run_bass_kernel_spmd(nc, in_maps, core_ids=list(range(8)), trace=trace)
    if trace:
        _CACHE["last_result"] = res
    return np.stack(
        [_unpack_out(res.results[b]["out"]) for b in range(8)]
    ).astype(np.float32)


# revision 51
# speedup vs baseline: 1.0645x; 1.0238x over previous
"""Trainium2 Bass kernel for PixelUnshuffle->MHA->PixelShuffle (nn_Attention).

Reference computation (per batch element, 8 batch elements data-parallel
across 8 NeuronCores):
  x [64, 256, 256] --PixelUnshuffle(8)--> tokens [N=1024, C=4096]
  qkv = tokens @ W_qkv            [1024, 768]
  4-head attention (d=64), softmax over tokens
  y = attn_out @ W_out + b_out    [1024, 4096]
  --PixelShuffle(8)--> [64, 256, 256]

v4 structure: the exp-bound attention stage is dissolved into the
projection stages so the scalar engine's exp stream (the old stage-2
floor, ~37us) hides under PE matmul work:

  Phase A: qk projections for both token windows (PE-dense).  The
    window-0 attention logits (dotsT = kT x qT) + exps for n-half 0,
    m-chunks 0-3 are interleaved into the window-1 qk pass.
  Phase B: v projections for both windows with ALL remaining dots+exp
    pairs interleaved (paced ~1 pair per 1.2us of PE work so the
    in-order PE queue never stalls on the dt PSUM double-buffer).  The
    exp outputs (ed tiles, bf16) are buffered in SBUF (64KB/partition).
  Phase C: attn@v runs as a dense PE burst per (n-half, head-pair)
    block (no exp dependency left), each block's 1/Z normalize chain
    (SBUF redistribute DMA + reciprocal + DRAM-round-trip broadcast)
    overlaps later blocks, and the output projection starts as soon as
    outT[0] is normalized.

Attention math is unchanged from v3: dotsT[m, n] summed-token-on-
partitions, exp on scalar (the only exp engine), a ones column in v
accumulating the softmax denominator Z for free (row 64 of oaug).

Startup: the first x tile is split into 8 per-r2 SLICE TILES (tile
dependency tracking is tile-granular) so the first real matmul needs
only 128KB of x + one 256KB weight chunk (~11us) instead of the whole
1MB tile.  All queues are loaded in consumption order; aggregate early
HBM bandwidth is ~300 GB/s (8-core contention), which bounds the qk-w0
pass; the r2 consumption order [0,4,1,5,2,6,3,7] alternates the
scalar/gpsimd weight queues to match arrival.

Layout/host packing identical to v3: x pre-packed+pre-cast bf16 into
[w*4+cg, p, r2*512 + hh*32 + ww] tiles; W_qkv split qk/v columns in
exact SBUF chunk layout; output written as raw [nq, ct, p, hq, ww, r2]
bf16 tiles and pixel-shuffled + upcast on the host.
"""

import sys

if "/opt/trn_rl_repo" not in sys.path:
    sys.path.insert(0, "/opt/trn_rl_repo")

import os

import ml_dtypes
import numpy as np

import concourse.bass as bass
from concourse import bacc, mybir, tile
from concourse.bass_utils import run_bass_kernel_spmd

F32 = mybir.dt.float32
BF16 = mybir.dt.bfloat16

SCALE = 0.125  # DIM_HEAD ** -0.5

_CACHE = {}

R2_ORDER = [0, 4, 1, 5, 2, 6, 3, 7]


def _build(zero_bias=False):
    nc = bacc.Bacc("TRN2", target_bir_lowering=False, debug=False, num_devices=8)

    # x pre-packed on host: [w*4+cg, p, r2*512 + hh*32 + ww]
    x_d = nc.dram_tensor("x", [8, 128, 4096], BF16, kind="ExternalInput").ap()
    # W_qkv split on host into qk columns and v columns, each pre-packed in
    # exact SBUF chunk layout so every load is one fully contiguous DMA
    wq_d = nc.dram_tensor("W_qkv", [8, 2, 128, 1024], BF16, kind="ExternalInput").ap()
    wqv_d = nc.dram_tensor("W_qkv_v", [8, 2, 128, 512], BF16, kind="ExternalInput").ap()
    wo_d = nc.dram_tensor("W_out", [256, 4096], BF16, kind="ExternalInput").ap()
    b_d = nc.dram_tensor("b_out", [4096], F32, kind="ExternalInput").ap()
    # raw output tiles: [(nq*4+ct)*2+half, p, r2l*256 + hq*32 + ww] with
    # r2 = half*4 + r2l; host pixel-shuffles
    out_d = nc.dram_tensor("out", [32, 128, 1024], BF16, kind="ExternalOutput").ap()

    zrc_d = nc.dram_tensor("zr_scratch", [4, 1024], BF16).ap()

    def dram_ap(base, off, pattern):
        return bass.AP(tensor=base.tensor, offset=base.offset + off, ap=pattern)

    with tile.TileContext(nc) as tc:
        _build_tiled(nc, tc, x_d, wq_d, wqv_d, wo_d, b_d, out_d, zrc_d, dram_ap, zero_bias)
    nc.compile()
    return nc


def _build_tiled(nc, tc, x_d, wq_d, wqv_d, wo_d, b_d, out_d, zrc_d, dram_ap, zero_bias=False):
    from contextlib import ExitStack

    with ExitStack() as ctx:
        pers = ctx.enter_context(tc.tile_pool(name="pers", bufs=1))
        edp = ctx.enter_context(tc.tile_pool(name="edp", bufs=1))
        xwp = ctx.enter_context(tc.tile_pool(name="xw", bufs=1))
        wqvp = ctx.enter_context(tc.tile_pool(name="wqv", bufs=1))
        # dt PSUM pool spans phases A+B only; closed before phase C so its
        # 4 banks are free for oaug/y_ps (8-bank limit)
        dt_stack = ExitStack()
        psDT = dt_stack.enter_context(tc.tile_pool(name="psdt", bufs=1, space="PSUM"))

        # z-chain pools are bound at phase-B entry (zps lives in the B
        # PSUM pool; see z_chain)
        zpool = [None]

        # ---- persistent tiles ----
        # qkT[d-part, ot, n] : ot 0,1 = q dims 0..128,128..256; ot 2,3 = k
        qkT = pers.tile([128, 4, 1024], BF16)
        # v[m-part, mc, h, d] bf16
        v_sb = pers.tile([128, 8, 4, 64], BF16)
        # outT[i-part, ic, n-half] split per nh for fine-grained stage-3 deps
        outT = [pers.tile([128, 2, 512], BF16, name=f"outT{nh}") for nh in range(2)]
        # bias[c-part, r2, cg]
        bias_sb = pers.tile([128, 8, 4], F32)
        # ones column for the Z partition-reduce matmuls
        ones_sb = pers.tile([128, 4], BF16)
        nc.vector.memset(ones_sb[:], 1.0)

        # preload the exp activation table off the critical path
        et_in = pers.tile([64, 16], F32)
        et_out = pers.tile([64, 16], F32)
        nc.vector.memset(et_in[:], 0.0)
        nc.scalar.activation(
            et_out[:], et_in[:], mybir.ActivationFunctionType.Exp, scale=SCALE
        )

        # ---- attention logits + exp, buffered in SBUF until phase C ----
        # Alongside each block's exps, vector accumulates edsum = sum_mc ed
        # so Z (softmax denominator) is a tiny matmul + reciprocal chain
        # that completes DURING phase B — by the time attn@v runs, 1/Z is
        # already broadcast in SBUF and normalize is one vector multiply.
        eds = {}
        edsums = {}
        zbcs = {}

        def dots_exp(nh, hp, mc):
            dts = psDT.tile(
                [128, 2, 512], F32, tag="dt", bufs=2, name=f"dt_{nh}_{hp}_{mc}"
            )
            for h2 in range(2):
                b = h2 * 64
                nc.tensor.matmul(
                    dts[:, h2, :],
                    qkT[b : b + 64, 2 + hp, mc * 128 : (mc + 1) * 128],
                    qkT[b : b + 64, hp, nh * 512 : (nh + 1) * 512],
                    start=True,
                    stop=True,
                )
            ed = edp.tile(
                [128, 2, 512], BF16, tag="ed", bufs=32, name=f"ed_{nh}_{hp}_{mc}"
            )
            nc.scalar.activation(
                ed[:].rearrange("p a b -> p (a b)"),
                dts[:].rearrange("p a b -> p (a b)"),
                mybir.ActivationFunctionType.Exp,
                scale=SCALE,
            )
            eds[(nh, hp, mc)] = ed
            if mc == 0:
                # edsum lives in the wqv pool (room freed by ed's pool so
                # stage-3's y_t can triple-buffer)
                es = wqvp.tile(
                    [128, 2, 512], BF16, tag="edsum", bufs=4, name=f"es_{nh}_{hp}"
                )
                nc.vector.tensor_copy(es[:], ed[:])
                edsums[(nh, hp)] = es
            else:
                es = edsums[(nh, hp)]
                nc.vector.tensor_add(es[:], es[:], ed[:])
            if mc == 7:
                z_chain(nh, hp)

        def z_chain(nh, hp):
            # only ever called from phase B (mc==7 exps all live there), so
            # the znp pool and zpool[0] (B PSUM pool) exist by the first call
            slot = nh * 2 + hp
            eA = (nc.sync, nc.gpsimd)[slot % 2]
            es = edsums[(nh, hp)]
            zps = zpool[0].tile(
                [1, 2, 512], F32, tag="zps", bufs=1, name=f"zps_{slot}"
            )
            for h2 in range(2):
                nc.tensor.matmul(
                    zps[:, h2, :], ones_sb[:, 0:1], es[:, h2, :],
                    start=True, stop=True,
                )
            # small vector copy off PSUM so the zps bank (and the B PSUM
            # pool at close) frees without waiting a DMA round trip
            zsb = znp.tile([1, 2, 512], F32, tag="zsb", bufs=1, name=f"zsb_{slot}")
            nc.vector.tensor_copy(zsb[:], zps[:])
            z64 = znp.tile([64, 16], F32, tag="z64", bufs=2)
            eA.dma_start(out=z64[:], in_=zsb[0:1, :, :])
            z64r = znp.tile([64, 16], F32, tag="z64r", bufs=2)
            nc.vector.reciprocal(z64r[:], z64[:])
            # bf16 for the DRAM round trip + broadcast tile (1/Z at 0.4%
            # rounding is far inside the error budget; halves the SBUF and
            # broadcast-DMA cost)
            z64rb = znp.tile([64, 16], BF16, tag="z64rb", bufs=2)
            nc.vector.tensor_copy(z64rb[:], z64r[:])
            eA.dma_start(
                out=zrc_d[slot, :].rearrange("(a b) -> a b", a=64),
                in_=z64rb[:],
            )
            # 1/Z broadcast: partitions 0-63 get the h2=0 row, 64-127 the
            # h2=1 row, matching the col-tiled av output layout
            zbc = znp.tile([128, 512], BF16, tag="zbc", bufs=4, name=f"zbc_{slot}")
            eA.dma_start(
                out=zbc[0:64, :],
                in_=dram_ap(zrc_d, slot * 1024, [[0, 64], [1, 512]]),
            )
            eA.dma_start(
                out=zbc[64:128, :],
                in_=dram_ap(zrc_d, slot * 1024 + 512, [[0, 64], [1, 512]]),
            )
            zbcs[(nh, hp)] = zbc

        # ---- x tiles: w0 tiles as two r2-half tiles (4KB rows keep the
        # DMA queues at full rate; tile-granular deps gate only 512KB),
        # w1 tiles whole.  xtbs[(w,cg)] is a list of 8 [128,16,32] views.
        xtbs = {}

        def load_x_quarter(w, cg, quarter, eng):
            # only for the startup-critical first tile: 256KB granularity
            # lets the first matmul fire ~2us earlier
            t = xwp.tile(
                [128, 2, 16, 32], BF16, tag="xtb0q", bufs=4,
                name=f"xq_{w}_{cg}_{quarter}",
            )
            eng.dma_start(
                out=t[:],
                in_=dram_ap(
                    x_d,
                    (w * 4 + cg) * 128 * 4096 + quarter * 1024,
                    [[4096, 128], [1, 1024]],
                ),
            )
            xtbs.setdefault((w, cg), [None] * 8)
            for r2l in range(2):
                xtbs[(w, cg)][quarter * 2 + r2l] = t[:, r2l]

        def load_x_half(w, cg, half, eng):
            # w0 halves and w1 halves use separate tags: a shared tag would
            # make a w1-half DMA wait on a w0 buffer that only frees in
            # phase B — after qk-w1 already needs the data (deadlock)
            t = xwp.tile(
                [128, 4, 16, 32], BF16, tag=f"xtb{w}h", bufs=8 if w == 0 else 2,
                name=f"xh_{w}_{cg}_{half}",
            )
            eng.dma_start(
                out=t[:],
                in_=dram_ap(
                    x_d,
                    (w * 4 + cg) * 128 * 4096 + half * 2048,
                    [[4096, 128], [1, 2048]],
                ),
            )
            xtbs.setdefault((w, cg), [None] * 8)
            for r2l in range(4):
                xtbs[(w, cg)][half * 4 + r2l] = t[:, r2l]

        def load_x_whole(w, cg, eng):
            t = xwp.tile(
                [128, 8, 16, 32], BF16, tag="xtb1", bufs=3, name=f"xtb_{w}_{cg}"
            )
            eng.dma_start(
                out=t[:],
                in_=dram_ap(x_d, (w * 4 + cg) * 128 * 4096, [[4096, 128], [1, 4096]]),
            )
            xtbs[(w, cg)] = [t[:, r2] for r2 in range(8)]

        # wqv chunks live in their own pool (consumed in phase B)
        wqv_t = {}

        def load_wq_v(r2, i, eng):
            t = wqvp.tile([128, 2, 256], BF16, name=f"wqv_{r2}_{i}")
            eng.dma_start(
                out=t[:],
                in_=dram_ap(wqv_d, (r2 * 2 + i) * 65536, [[512, 128], [1, 512]]),
            )
            wqv_t[(r2, i)] = t

        # =========================== phase A ===========================
        # qk projection, both windows; dots+exp for (nh0, mc0-3)
        # interleaved into the w1 pass.
        with (
            tc.tile_pool(name="wq", bufs=1) as wqp,
            tc.tile_pool(name="ps1", bufs=1, space="PSUM") as ps1,
        ):
            wqk_t = {}

            def load_wq_qk(r2, i, eng):
                t = wqp.tile([128, 2, 512], BF16, name=f"wqk_{r2}_{i}")
                eng.dma_start(
                    out=t[:],
                    in_=dram_ap(wq_d, (r2 * 2 + i) * 131072, [[1024, 128], [1, 1024]]),
                )
                wqk_t[(r2, i)] = t

            # PE warmup: dummy matmuls so HAM is at 2.4 GHz when the first
            # real matmul arrives (~11us)
            warm = wqp.tile([128, 256], BF16)
            nc.vector.memset(warm[:], 0.0)
            warm_ps = ps1.tile([128, 512], F32, tag="qk0", bufs=1)
            for i in range(18):
                nc.tensor.matmul(
                    warm_ps[:, 0:256], warm[:, 0:128], warm[:], start=True, stop=True
                )

            # ---- DMA staging, consumption-ordered per queue (each queue
            # sustains only ~90-110 GB/s early with 8-core HBM contention;
            # every item below is placed so it lands just before the qk
            # matmul stream consumes it). ----
            # sync:   x00A w4 w5 x01A x01B x02A x02B x12 | wo[0] @B
            # scalar: wqk i0 r0-3, wqk i1 r0-3, x10, x13 | wqv r0-3 @mid-A
            # gpsimd: x00B w6 w7, i1 r4-7, x03A x03B x11 | wqv r4-7, bias
            load_x_half(0, 0, 0, nc.sync)
            load_wq_qk(0, 0, nc.scalar)
            load_x_half(0, 0, 1, nc.gpsimd)
            load_wq_qk(1, 0, nc.scalar)
            load_wq_qk(4, 0, nc.sync)
            load_wq_qk(6, 0, nc.gpsimd)
            load_wq_qk(2, 0, nc.scalar)
            load_wq_qk(5, 0, nc.sync)
            load_wq_qk(7, 0, nc.gpsimd)
            load_wq_qk(3, 0, nc.scalar)
            for r2 in (0, 1, 2, 3):
                load_wq_qk(r2, 1, nc.scalar)
            for r2 in (4, 5, 6, 7):
                load_wq_qk(r2, 1, nc.gpsimd)
            load_x_half(0, 1, 0, nc.sync)
            load_x_half(0, 1, 1, nc.sync)
            load_x_half(0, 2, 0, nc.sync)
            load_x_half(0, 2, 1, nc.sync)
            load_x_half(0, 3, 0, nc.gpsimd)
            load_x_half(0, 3, 1, nc.gpsimd)
            load_x_half(1, 0, 0, nc.scalar)
            load_x_half(1, 0, 1, nc.scalar)
            load_x_whole(1, 2, nc.sync)
            load_x_whole(1, 1, nc.gpsimd)
            load_x_whole(1, 3, nc.scalar)

            # ---- qk matmul passes ----
            # per-cg r2 consumption order matched to DMA arrival order
            W0_ORDERS = [
                [0, 1, 6, 4, 2, 7, 5, 3],
                [0, 4, 1, 5, 2, 6, 3, 7],
                [0, 4, 1, 5, 2, 6, 3, 7],
                [0, 4, 1, 5, 2, 6, 3, 7],
            ]
            W1_ORDERS = [list(range(8))] * 4

            def qk_window(w, dots_list):
                # dots_list: (nh, hp, mc) tuples to interleave, 2 per cg
                orders = W0_ORDERS if w == 0 else W1_ORDERS
                qks = [
                    ps1.tile([128, 512], F32, tag=f"qk{ot}", bufs=1, name=f"qk_{w}_{ot}")
                    for ot in range(4)
                ]
                for cg in range(4):
                    xv = xtbs[(w, cg)]
                    for ri, r2 in enumerate(orders[cg]):
                        first = cg == 0 and ri == 0
                        last = cg == 3 and ri == 7
                        for ot in range(4):
                            nc.tensor.matmul(
                                qks[ot][:],
                                wqk_t[(r2, cg // 2)][
                                    :, cg % 2, ot * 128 : (ot + 1) * 128
                                ],
                                xv[r2][:],
                                start=first,
                                stop=last,
                            )
                    for d in dots_list[2 * cg : 2 * cg + 2]:
                        dots_exp(*d)
                # k evacuations (ot 2,3) first: earliest dots need them
                for ot in (2, 3, 0, 1):
                    dst = qkT[:, ot, w * 512 : (w + 1) * 512]
                    if ot % 2 == 0:
                        nc.scalar.copy(dst, qks[ot][:])
                    else:
                        nc.vector.tensor_copy(dst, qks[ot][:])

            qk_window(0, [])
            # wqv + bias issues between the windows: by now each engine's
            # ring has drained, and their data is needed only in phase B
            for i in range(2):
                for r2 in (0, 1, 2, 3):
                    load_wq_v(r2, i, nc.scalar)
                for r2 in (4, 5, 6, 7):
                    load_wq_v(r2, i, nc.gpsimd)
            # host pre-arranges b_out as [p, r2, cg] so this is a flat copy
            nc.gpsimd.dma_start(
                out=bias_sb[:],
                in_=dram_ap(b_d, 0, [[32, 128], [4, 8], [1, 4]]),
            )
            qk_window(1, [(0, 0, 0), (0, 0, 1), (0, 0, 2), (0, 0, 3),
                          (0, 1, 0), (0, 1, 1), (0, 1, 2), (0, 1, 3)])

        # =========================== phase B ===========================
        # v projection, both windows, with the remaining 24 dots+exp
        # pairs paced in (~1 per 11 v-matmuls ~= 1.2us of PE work).
        # W_out loads here (deadline: phase C), in the SBUF slot wqk freed.
        wop = ctx.enter_context(tc.tile_pool(name="wop", bufs=1))
        # z-chain tiles live B..C
        znp = ctx.enter_context(tc.tile_pool(name="znp", bufs=1))
        wo_sb = wop.tile([128, 2, 4096], BF16)  # [i-part, ic, c_perm]
        nc.sync.dma_start(
            out=wo_sb[:, 0, :],
            in_=dram_ap(wo_d, 0, [[4096, 128], [1, 4096]]),
        )
        nc.gpsimd.dma_start(
            out=wo_sb[:, 1, :],
            in_=dram_ap(wo_d, 524288, [[4096, 128], [1, 4096]]),
        )
        rem = (
            [(0, 0, mc) for mc in range(4, 8)]
            + [(0, 1, mc) for mc in range(4, 8)]
            + [(1, 0, mc) for mc in range(8)]
            + [(1, 1, mc) for mc in range(8)]
        )
        with tc.tile_pool(name="psv", bufs=1, space="PSUM") as psV:
            zpool[0] = psV
            mm_ctr = [0]
            di = [0]

            def maybe_dots():
                # insert the next dots pair every 11 v-matmuls
                if di[0] < len(rem) and mm_ctr[0] >= 11:
                    mm_ctr[0] = 0
                    dots_exp(*rem[di[0]])
                    di[0] += 1

            # s-outer: one token-chunk accumulation group at a time, so the
            # v pass needs only 2 PSUM banks (zps + dt take the rest)
            for w in range(2):
                for s in range(4):
                    vps = psV.tile(
                        [128, 256], F32, tag=f"v{s % 2}", bufs=1, name=f"v_{w}_{s}"
                    )
                    for cg in range(4):
                        xv = xtbs[(w, cg)]
                        for r2 in range(8):
                            nc.tensor.matmul(
                                vps[:],
                                xv[r2][:, 4 * s : 4 * s + 4, :],
                                wqv_t[(r2, cg // 2)][:, cg % 2, :],
                                start=(cg == 0 and r2 == 0),
                                stop=(cg == 3 and r2 == 7),
                            )
                            mm_ctr[0] += 1
                            maybe_dots()
                    nc.vector.tensor_copy(
                        v_sb[:, 4 * w + s, :, :],
                        vps[:].rearrange("p (h d) -> p h d", h=4),
                    )
            # flush any unpaced dots pairs (shouldn't happen: 256 MMs/11 > 23)
            while di[0] < len(rem):
                dots_exp(*rem[di[0]])
                di[0] += 1
            # keep-alive matmuls: the B->C pool-close barrier idles the PE
            # ~3-4us (last exp gates the dt release); these run through it
            # so HAM stays at 2.4 GHz into the av burst
            ka = psV.tile([128, 256], F32, tag="v0", bufs=1, name="ka")
            for i in range(16):
                nc.tensor.matmul(
                    ka[0:1, :], ones_sb[:, 0:1], qkT[:, 0, 0:256],
                    start=True, stop=True,
                )

        # dt banks free for phase C once the last exp has read them
        dt_stack.close()

        # =========================== phase C ===========================
        # av bursts + per-block normalize + output projection.
        ps3 = [None]
        with (
            tc.tile_pool(name="s3b", bufs=1) as s3,
            tc.tile_pool(name="ps3", bufs=1, space="PSUM") as ps3p,
        ):
            ps3[0] = ps3p

            def av_block(nh, hp):
                # col-tiled av: h2=0 writes PSUM partitions 0-63, h2=1
                # writes 64-127 — disjoint col groups run concurrently, so
                # one (mc) step costs one N=512 matmul slot, and the output
                # lands directly in outT's partition layout
                # the accumulator borrows the stage-3 PSUM slot that the
                # (nh, hp)-matching output group will use next: no pool
                # swap between av and the output projection, so the PE
                # rolls straight from av into stage-3 matmuls
                slot = nh * 2 + hp
                oaug4 = ps3[0].tile(
                    [128, 4, 256], F32, tag=f"yps{slot // 2}{slot % 2}", bufs=1,
                    name=f"oaug_{nh}_{hp}",
                )
                oaug = oaug4[:, 0:2, :].rearrange("p a b -> p (a b)")
                for mc in range(8):
                    edc = eds[(nh, hp, mc)]
                    for h2 in range(2):
                        nc.tensor.matmul(
                            oaug[64 * h2 : 64 * h2 + 64],
                            v_sb[:, mc, 2 * hp + h2, :],
                            edc[:, h2, :],
                            start=(mc == 0),
                            stop=(mc == 7),
                        )
                # normalize: 1/Z was pre-broadcast during phase B — one mul
                nc.vector.tensor_mul(
                    outT[nh][:, hp, :], oaug[:, :], zbcs[(nh, hp)][:]
                )

            s3_idx = [0]

            def stage3_group(nqp, ct, half):
                # two nq tiles per group share every W_out LDWEIGHTS (one
                # load feeds two N=256 matmuls, fully hiding the weight-
                # load tax); 2 groups rotate through the 8 PSUM banks
                rot = s3_idx[0] % 2
                s3_idx[0] += 1
                nqs = (2 * nqp, 2 * nqp + 1)
                y_ps = {
                    nq: ps3[0].tile(
                        [128, 4, 256], F32, tag=f"yps{rot}{nq % 2}", bufs=1,
                        name=f"yps_{nq}_{ct}_{half}",
                    )
                    for nq in nqs
                }
                for r2l in range(4):
                    r2 = half * 4 + r2l
                    for ic in range(2):
                        for nq in nqs:
                            nc.tensor.matmul(
                                y_ps[nq][:, r2l, :],
                                wo_sb[
                                    :,
                                    ic,
                                    r2 * 512 + ct * 128 : r2 * 512 + (ct + 1) * 128,
                                ],
                                outT[nq // 2][
                                    :, ic, (nq % 2) * 256 : (nq % 2 + 1) * 256
                                ],
                                start=(r2l % 2 == 0 and ic == 0),
                                stop=(r2l % 2 == 1 and ic == 1),
                            )
                for nq in nqs:
                    y_t = s3.tile(
                        [128, 4, 256], BF16, tag=f"yt{nq % 2}", bufs=3,
                        name=f"yt_{nq}_{ct}_{half}",
                    )
                    if zero_bias:
                        # evacuation split across vector+scalar (~0.7us
                        # each < the ~0.9us of matmul work per tile);
                        # gpsimd has no PSUM port
                        nc.vector.tensor_copy(y_t[:, 0:2, :], y_ps[nq][:, 0:2, :])
                        nc.scalar.copy(y_t[:, 2:4, :], y_ps[nq][:, 2:4, :])
                    else:
                        bias_bc4 = bias_sb[
                            :, half * 4 : (half + 1) * 4, ct
                        ][:, :, None].broadcast_to([128, 4, 256])
                        nc.vector.tensor_add(
                            y_t[:, 0:2, :], y_ps[nq][:, 0:2, :], bias_bc4[:, 0:2, :]
                        )
                        nc.vector.tensor_add(
                            y_t[:, 2:4, :], y_ps[nq][:, 2:4, :], bias_bc4[:, 2:4, :]
                        )
                    # every tile drains as two 128KB DMAs (partition-split
                    # keeps 2KB contiguous rows) on sync/gpsimd only —
                    # scalar's evacuation stream must never stall on a
                    # ring-full DMA issue
                    for qq in range(2):
                        deng = (nc.sync, nc.gpsimd)[(nq + qq) % 2]
                        deng.dma_start(
                            out=dram_ap(
                                out_d,
                                ((nq * 4 + ct) * 2 + half) * 128 * 1024
                                + qq * 64 * 1024,
                                [[1024, 64], [1, 1024]],
                            ),
                            in_=y_t[64 * qq : 64 * qq + 64, :, :],
                        )

            av_block(0, 0)
            av_block(0, 1)
            av_block(1, 0)
            av_block(1, 1)
            for nqp in range(2):
                for ct in range(4):
                    for half in range(2):
                        stage3_group(nqp, ct, half)


def _get_nc(zero_bias=False):
    key = f"nc_zb{int(zero_bias)}"
    if key not in _CACHE:
        _CACHE[key] = _build(zero_bias=zero_bias)
    return _CACHE[key]


def _prep_weights(W_qkv, W_out, b_out):
    wq_perm = (
        W_qkv.reshape(64, 8, 8, 768).transpose(2, 0, 1, 3).reshape(4096, 768)
    )
    # split qk vs v columns and pack each in exact SBUF chunk layout
    # [r2, i(cg pair), p, (cgl, cols)] so every device load is one fully
    # contiguous DMA: rows within an (r2, i) chunk are (cgl*128 + p)
    def pack(cols):
        n = cols.shape[1]
        t = cols.reshape(8, 2, 2, 128, n)       # [r2, i, cgl, p, o]
        t = t.transpose(0, 1, 3, 2, 4)          # [r2, i, p, cgl, o]
        return np.ascontiguousarray(
            t.reshape(8, 2, 128, 2 * n)
        ).astype(ml_dtypes.bfloat16)

    wq_qk = pack(wq_perm[:, 0:512])
    wq_v = pack(wq_perm[:, 512:768])
    wo_perm = np.ascontiguousarray(
        W_out.reshape(256, 64, 8, 8).transpose(0, 3, 1, 2).reshape(256, 4096)
    ).astype(ml_dtypes.bfloat16)
    # b_perm[r2*512 + c0*8 + r1] = b_out[c0*64 + r1*8 + r2], then laid out
    # [p, r2, cg] where p = (c0 % 16)*8 + r1, cg = c0 // 16
    b_perm = b_out.reshape(64, 8, 8).transpose(2, 0, 1).reshape(4096)
    b_perm = np.ascontiguousarray(
        b_perm.reshape(8, 4, 128).transpose(2, 0, 1).reshape(4096)
    ).astype(np.float32)
    return wq_qk, wq_v, wo_perm, b_perm


def _pack_x(xb):
    # xb [64, 256, 256] f32 -> [w*4+cg, p=(c0%16)*8+r1, r2*512+hh*32+ww] bf16
    # x[c0, (w*16+hh)*8 + r1, ww*8 + r2]
    t = xb.astype(ml_dtypes.bfloat16)
    t = t.reshape(4, 16, 2, 16, 8, 32, 8)  # [cg, c0l, w, hh, r1, ww, r2]
    t = t.transpose(2, 0, 1, 4, 6, 3, 5)   # [w, cg, c0l, r1, r2, hh, ww]
    return np.ascontiguousarray(t.reshape(8, 128, 4096))


def _unpack_out(raw):
    # raw [32, 128, 1024] = [(nq*4+ct)*2+half, (c0%16)*8+r1, r2l*256+hq*32+ww]
    # with r2 = half*4 + r2l -> y[c0, (nq*8+hq)*8 + r1, ww*8 + r2]
    t = np.asarray(raw).reshape(4, 4, 2, 16, 8, 4, 8, 32)
    # [nq, ct, half, c0l, r1, r2l, hq, ww]
    t = t.transpose(1, 3, 0, 6, 4, 7, 2, 5)  # [ct, c0l, nq, hq, r1, ww, half, r2l]
    return t.reshape(64, 256, 256)


def kernel(x, W_qkv, W_out, b_out):
    nc = _get_nc(zero_bias=not np.any(np.asarray(b_out)))
    wq_qk, wq_v, wo_perm, b_perm = _prep_weights(
        np.asarray(W_qkv, dtype=np.float32),
        np.asarray(W_out, dtype=np.float32),
        np.asarray(b_out, dtype=np.float32),
    )

    in_maps = [
        {
            "x": _pack_x(np.asarray(x[b], dtype=np.float32)),
            "W_qkv": wq_qk,
            "W_qkv_v": wq_v,
            "W_out": wo_perm,
            "b_out": b_perm,
        }
        for b in range(8)
    ]
    trace = bool(int(os.environ.get("BENCH_TRACE", "0")))
    if trace:
        try:  # tracing needs the NTFF hook shim (see test.py); degrade if absent
            from antenv.axon_hooks import get_axon_ntff_profile_hook  # noqa: F401
        except ImportError:
            trace = False
    res = run_bass_kernel_spmd(nc, in_maps, core_ids=list(range(8)), trace=trace)
    if trace:
        _CACHE["last_result"] = res
    return np.stack(
        [_unpack_out(res.results[b]["out"]) for b in range(8)]
    ).astype(np.float32)
